# revision 20
# baseline (speedup 1.0000x reference)
"""BERT-base forward (B=16, S=512, D=768, H=12, L=12) on 8 Trainium2 NeuronCores.

Sharding: data-parallel over batch — each core runs 2 sequences (1024 tokens)
with a full replica of the weights. No collectives.

Device layout: "transposed activations" — activations live as x^T [D, tokens]
(features on SBUF partitions, tokens on the free dim), so every linear layer is
out^T = W^T.T @ x^T with the (host-pre-tiled) weight as the stationary operand
and 512-token chunks as the moving operand.

v2 structure (vs the v1 baseline):
- V is computed by a "flipped" GEMM (x^T chunks stationary, V-weight moving),
  producing V directly in [token, head_dim] orientation — no DMA transposes.
- QK logits use PE row-tiling (two 64-contraction head matmuls concurrent via
  tile_position (0,0)/(64,0)); attn@V and the ones-denominator matmuls use PE
  col-tiling ((0,0)/(0,64)) so both heads of a pair run concurrently.
- LayerNorm: ln scale is folded into the downstream GEMM weights on the host,
  so the bf16 GEMM shadow is just (x-mean)*rstd; rstd = exp(-0.5*ln(var+eps))
  keeps all transcendentals in the natural_log_exp ACT table set (shared with
  attention's exp) — only gelu forces table swaps, prefetched via dummy ops.
- Attention softmax normalization reads PSUM directly on DVE (reciprocal +
  multiply), no ACT identity copies.
"""
import sys
sys.path.insert(0, '/opt/trn_rl_repo')

import numpy as np
import ml_dtypes
import concourse.bass as bass
import concourse.tile as tile
from concourse import bacc, mybir
from concourse.bass_utils import run_bass_kernel_spmd

# Model shapes (hardcoded)
V = 30522
S = 512
D = 768
H = 12
L = 12
F = 3072
B = 16
HD = 64
EPS = 1e-12
SCALE = HD ** (-0.5)

NCORES = 8
B_LOC = B // NCORES          # 2 sequences per core
T = B_LOC * S                # 1024 tokens per core
KD = D // 128                # 6 k-tiles over D
QKM = 2 * D // 128           # 12 m-tiles for q,k
FM = F // 128                # 24 m-tiles over mlp hidden
PAIRS = H // 2               # 6 head-pairs
TCH = 512                    # token chunk for all GEMMs
NT = T // TCH                # 2
NC = T // 128                # 8 128-token chunks

F32 = mybir.dt.float32
F32R = mybir.dt.float32r
BF16 = mybir.dt.bfloat16
I32 = mybir.dt.int32
AF = mybir.ActivationFunctionType
OP = mybir.AluOpType

_CACHED_NC = None
DEBUG = False


def _host_tile_weight(w_t):
    """w_t: [dout, din] torch-Linear weight. Returns [m_tiles, 128, din] where
    slice [m] is (w_t.T)[:, m*128:(m+1)*128] laid out partition-major."""
    dout, din = w_t.shape
    m_tiles, k_tiles = dout // 128, din // 128
    a = np.ascontiguousarray(w_t.T)                      # [din, dout]
    a = a.reshape(k_tiles, 128, m_tiles, 128)            # [k, p, m, w]
    a = np.ascontiguousarray(a.transpose(2, 1, 0, 3))    # [m, p, k, w]
    return a.reshape(m_tiles, 128, din).astype(ml_dtypes.bfloat16)


def build_nc(n_layers=L):
    nc = bacc.Bacc("TRN2", target_bir_lowering=False, debug=False)

    def din(name, shape, dt=BF16):
        return nc.dram_tensor(name, shape, dt, kind="ExternalInput").ap()

    nl = max(1, n_layers)
    tokens = din("tokens", [T], I32)
    tok_emb = din("tok_emb", [V, D], F32R)
    possent = din("possent", [S, D], F32)
    embw = din("embw", [D], F32)
    embb = din("embb", [D], F32)
    wqk = din("wqk", [nl, QKM, 128, D])
    bqk = din("bqk", [nl, 2 * D], F32)
    wvT = din("wvT", [nl, 128, KD, D])
    bv = din("bv", [nl, D], F32)
    wproj = din("wproj", [nl, KD, 128, D])
    bproj = din("bproj", [nl, D], F32)
    w1 = din("w1", [nl, FM, 128, D])
    b1 = din("b1", [nl, F], F32)
    w2 = din("w2", [nl, KD, 128, F])
    b2 = din("b2", [nl, D], F32)
    ln1w = din("ln1w", [nl, D], F32)
    ln1b = din("ln1b", [nl, D], F32)
    ln2w = din("ln2w", [nl, D], F32)
    ln2b = din("ln2b", [nl, D], F32)
    wpool = din("wpool", [KD, 128, D], BF16)
    bpool = din("bpool", [D], F32)
    ident = din("ident", [128, 128], F32R)
    ones = din("ones", [128, 128], F32R)
    ones_bf = din("ones_bf", [128, 128], BF16)
    out = nc.dram_tensor("out", [B_LOC, D], F32, kind="ExternalOutput").ap()
    dbg = {}
    if DEBUG:
        dbg['v'] = nc.dram_tensor("dbg_v", [128, NC, D], BF16, kind="ExternalOutput").ap()
        dbg['aT'] = nc.dram_tensor("dbg_aT", [128, KD, T], BF16, kind="ExternalOutput").ap()
        dbg['x1'] = nc.dram_tensor("dbg_x1", [128, KD, T], F32R, kind="ExternalOutput").ap()
        dbg['xb1'] = nc.dram_tensor("dbg_xb1", [128, KD, T], BF16, kind="ExternalOutput").ap()
        dbg['h'] = nc.dram_tensor("dbg_h", [128, FM, T], BF16, kind="ExternalOutput").ap()
        dbg['xbf'] = nc.dram_tensor("dbg_xbf", [128, KD, T], BF16, kind="ExternalOutput").ap()
        dbg['qk'] = nc.dram_tensor("dbg_qk", [128, 2, T], BF16, kind="ExternalOutput").ap()
        dbg['expP'] = nc.dram_tensor("dbg_expP", [128, 4, S], BF16, kind="ExternalOutput").ap()
        dbg['psu0'] = nc.dram_tensor("dbg_psu0", [128, S], F32, kind="ExternalOutput").ap()
        dbg['psd0'] = nc.dram_tensor("dbg_psd0", [128, S], F32, kind="ExternalOutput").ap()
        dbg['rec'] = nc.dram_tensor("dbg_rec", [128, S], F32, kind="ExternalOutput").ap()

    with tile.TileContext(nc) as tc:
        _build_body(nc, tc, n_layers, tokens, tok_emb, possent, embw, embb,
                    wqk, bqk, wvT, bv, wproj, bproj, w1, b1, w2, b2,
                    ln1w, ln1b, ln2w, ln2b, wpool, bpool, ident, ones, ones_bf,
                    out, dbg)
    nc.compile()
    return nc


def _build_body(nc, tc, n_layers, tokens, tok_emb, possent, embw, embb,
                wqk, bqk, wvT, bv, wproj, bproj, w1, b1, w2, b2,
                ln1w, ln1b, ln2w, ln2b, wpool, bpool, ident, ones, ones_bf,
                out, dbg={}):
    from contextlib import ExitStack
    ctx = ExitStack()
    with ctx:
        consts = ctx.enter_context(tc.tile_pool(name="consts", bufs=1))
        xpool = ctx.enter_context(tc.tile_pool(name="xpool", bufs=1))
        qkpool = ctx.enter_context(tc.tile_pool(name="qkpool", bufs=3))
        wpool6 = ctx.enter_context(tc.tile_pool(name="wpool6", bufs=8))
        psum = ctx.enter_context(tc.tile_pool(name="psum", bufs=4, space="PSUM"))
        psum2 = ctx.enter_context(tc.tile_pool(name="psum2", bufs=2, space="PSUM"))

        ident_sb = consts.tile([128, 128], F32R)
        nc.sync.dma_start(ident_sb[:], ident)
        ones_sb = consts.tile([128, 128], F32R)
        nc.sync.dma_start(ones_sb[:], ones)
        onesb_sb = consts.tile([128, 128], BF16)
        nc.sync.dma_start(onesb_sb[:], ones_bf)
        eps_sb = consts.tile([128, 1], F32)
        nc.vector.memset(eps_sb[:], EPS)
        scratch = consts.tile([128, 8], F32)

        # residual stream x^T (fp32) + bf16 shadow (= (x-mu)*rstd, no w/b)
        xT = xpool.tile([128, KD, T], F32R)
        xTb = xpool.tile([128, KD, T], BF16)
        # attention output a^T (bf16: proj GEMM input)
        aT = xpool.tile([128, KD, T], BF16)
        # V in [token, dout] orientation: v_sb[p, c, j] = v[token c*128+p, j]
        v_sb = xpool.tile([128, NC, D], BF16)

        def ps1():
            return psum.tile([128, TCH], F32, tag="mm1", name="ps1")

        def ps2():
            return psum2.tile([128, 2, TCH], F32, tag="mm2", name="ps2")

        # ---------------- Embedding ----------------
        with tc.tile_pool(name="embp", bufs=2) as embp:
            for tt in range(T // 128):
                idx_sb = embp.tile([128, 1], I32, tag="idx")
                nc.sync.dma_start(idx_sb[:], tokens[tt * 128:(tt + 1) * 128, None])
                g_sb = embp.tile([128, D], F32R, tag="g")
                nc.gpsimd.indirect_dma_start(
                    out=g_sb[:], out_offset=None, in_=tok_emb,
                    in_offset=bass.IndirectOffsetOnAxis(ap=idx_sb[:, :1], axis=0))
                p_sb = embp.tile([128, D], F32, tag="p")
                prow = (tt * 128) % S
                nc.sync.dma_start(p_sb[:], possent[prow:prow + 128, :])
                nc.vector.tensor_add(g_sb[:], g_sb[:], p_sb[:])
                # LayerNorm over free dim (d): bn_stats in 2 subgroups of 384
                st_sb = embp.tile([128, 2, 6], F32, tag="st")
                gv = g_sb[:].rearrange("p (a b) -> p a b", a=2)
                for a in range(2):
                    nc.vector.bn_stats(st_sb[:, a, :], gv[:, a, :])
                mv = embp.tile([128, 2], F32, tag="mv")
                nc.vector.bn_aggr(mv[:], st_sb[:])
                sd = embp.tile([128, 1], F32, tag="sd")
                nc.scalar.activation(sd[:], mv[:, 1:2], AF.Sqrt, bias=eps_sb[:])
                nc.vector.reciprocal_approx_fast(sd[:], sd[:])
                nc.vector.tensor_scalar(g_sb[:], g_sb[:], mv[:, 0:1], sd[:],
                                        op0=OP.subtract, op1=OP.mult)
                # transpose (x-mu)*rstd into xT; w/b applied after in
                # transposed space (w is folded into l=0 GEMM weights for the
                # shadow; xT carries the full LN output for the residual)
                for k in range(KD):
                    pst = psum.tile([128, 128], F32R, tag="mm1", name="pst")
                    nc.tensor.transpose(pst[:], g_sb[:, k * 128:(k + 1) * 128],
                                        ident_sb[:])
                    nc.vector.tensor_copy(xT[:, k, tt * 128:(tt + 1) * 128], pst[:])
            nc.vector.tensor_copy(xTb[:], xT[:])
            embwk_sb = embp.tile([128, KD], F32, tag="embwk")
            nc.sync.dma_start(embwk_sb[:], embw.rearrange("(k p) -> p k", p=128))
            embbk_sb = embp.tile([128, KD], F32, tag="embbk")
            nc.sync.dma_start(embbk_sb[:], embb.rearrange("(k p) -> p k", p=128))
            for k in range(KD):
                nc.gpsimd.tensor_scalar(xT[:, k, :], xT[:, k, :],
                                        embwk_sb[:, k:k + 1], embbk_sb[:, k:k + 1],
                                        op0=OP.mult, op1=OP.add)

        # ---------------- Layer norm (transposed layout) ----------------
        def layer_norm_T(lw_sb, lb_sb, lnp, finalize):
            # stats via all-ones matmuls on the fp32 residual. Shadow xTb gets
            # (x-mu)*rstd (ln w folded into downstream weights, bias folded
            # into downstream GEMM bias on the host). xT gets the full LN
            # output off the critical path.
            for tch in range(NT):
                tsl = slice(tch * TCH, (tch + 1) * TCH)
                xs = xT[:, :, tsl]
                ps_s = ps1()
                for k in range(KD):
                    nc.tensor.matmul(ps_s[:], lhsT=ones_sb[:], rhs=xT[:, k, tsl],
                                     start=(k == 0), stop=(k == KD - 1))
                ps_q = ps1()
                for k in range(KD):
                    sq = lnp.tile([128, TCH], F32R, tag="sq")
                    if k < 3:
                        nc.vector.tensor_mul(sq[:], xT[:, k, tsl], xT[:, k, tsl])
                    else:
                        nc.gpsimd.tensor_mul(sq[:], xT[:, k, tsl], xT[:, k, tsl])
                    nc.tensor.matmul(ps_q[:], lhsT=ones_sb[:], rhs=sq[:],
                                     start=(k == 0), stop=(k == KD - 1))
                mean = lnp.tile([128, TCH], F32, tag="mean")
                nc.vector.tensor_scalar_mul(mean[:], ps_s[:], 1.0 / D)
                var = lnp.tile([128, TCH], F32, tag="var")
                nc.vector.tensor_mul(var[:], mean[:], mean[:])
                nc.vector.scalar_tensor_tensor(var[:], in0=ps_q[:], scalar=1.0 / D,
                                               in1=var[:], op0=OP.mult,
                                               op1=OP.subtract)
                # rstd = exp(-0.5*ln(var+eps)) — stays in the ln/exp ACT set
                r = lnp.tile([128, TCH], F32, tag="r")
                nc.scalar.activation(r[:], var[:], AF.Sqrt, bias=eps_sb[:])
                nc.vector.reciprocal_approx_fast(r[:], r[:])
                mean_bc4 = mean[:, None, :].to_broadcast([128, 4, TCH])
                mean_bc2 = mean[:, None, :].to_broadcast([128, 2, TCH])
                nc.vector.tensor_sub(xs[:, 0:4, :], xs[:, 0:4, :], mean_bc4)
                nc.gpsimd.tensor_sub(xs[:, 4:6, :], xs[:, 4:6, :], mean_bc2)
                r_bc4 = r[:, None, :].to_broadcast([128, 4, TCH])
                r_bc2 = r[:, None, :].to_broadcast([128, 2, TCH])
                nc.vector.tensor_mul(xs[:, 0:4, :], xs[:, 0:4, :], r_bc4)
                nc.gpsimd.tensor_mul(xs[:, 4:6, :], xs[:, 4:6, :], r_bc2)
                # bf16 shadow for the GEMMs (critical path ends here)
                nc.vector.tensor_copy(xTb[:, 0:3, tsl], xs[:, 0:3, :])
                nc.scalar.activation(xTb[:, 3:6, tsl], xs[:, 3:6, :],
                                     AF.Identity)
                if finalize:
                    # xT = shadow*w + b (full LN output for the residual)
                    for k in range(KD):
                        nc.gpsimd.tensor_scalar(xs[:, k, :], xs[:, k, :],
                                                lw_sb[:, k:k + 1],
                                                lb_sb[:, k:k + 1],
                                                op0=OP.mult, op1=OP.add)

        # ---------------- Layers ----------------
        lctx = ExitStack()
        hpool = lctx.enter_context(tc.tile_pool(name="hpool", bufs=1))
        wpool24 = lctx.enter_context(tc.tile_pool(name="wpool24", bufs=2))
        wvpool = lctx.enter_context(tc.tile_pool(name="wvpool", bufs=1))
        biasp = lctx.enter_context(tc.tile_pool(name="biasp", bufs=1))
        attnp = lctx.enter_context(tc.tile_pool(name="attnp", bufs=4))
        recp = lctx.enter_context(tc.tile_pool(name="recp", bufs=2))
        lnp_pool = lctx.enter_context(tc.tile_pool(name="lnp", bufs=2))
        h = hpool.tile([128, FM, T], BF16, tag="h")
        for l in range(n_layers):
            # per-layer bias/ln tiles
            bqk_sb = biasp.tile([128, QKM], F32, tag="bqk")
            nc.sync.dma_start(bqk_sb[:], bqk[l].rearrange("(m p) -> p m", p=128))
            bv_sb = biasp.tile([128, D], F32, tag="bv")
            nc.sync.dma_start(bv_sb[:], bv[l][None, :].to_broadcast([128, D]))
            bp_sb = biasp.tile([128, KD], F32, tag="bp")
            nc.sync.dma_start(bp_sb[:], bproj[l].rearrange("(m p) -> p m", p=128))
            b1_sb = biasp.tile([128, FM], F32, tag="b1")
            nc.sync.dma_start(b1_sb[:], b1[l].rearrange("(m p) -> p m", p=128))
            b2_sb = biasp.tile([128, KD], F32, tag="b2")
            nc.sync.dma_start(b2_sb[:], b2[l].rearrange("(m p) -> p m", p=128))
            l1w_sb = biasp.tile([128, KD], F32, tag="l1w")
            nc.sync.dma_start(l1w_sb[:], ln1w[l].rearrange("(k p) -> p k", p=128))
            l1b_sb = biasp.tile([128, KD], F32, tag="l1b")
            nc.sync.dma_start(l1b_sb[:], ln1b[l].rearrange("(k p) -> p k", p=128))
            l2w_sb = biasp.tile([128, KD], F32, tag="l2w")
            nc.sync.dma_start(l2w_sb[:], ln2w[l].rearrange("(k p) -> p k", p=128))
            l2b_sb = biasp.tile([128, KD], F32, tag="l2b")
            nc.sync.dma_start(l2b_sb[:], ln2b[l].rearrange("(k p) -> p k", p=128))

            # ---- V via flipped GEMM: v[tok, j] directly (no transposes) ----
            wv_sb = wvpool.tile([128, KD, D], BF16, tag="wv")
            nc.sync.dma_start(wv_sb[:], wvT[l])
            for c in range(NC):
                csl = slice(c * 128, (c + 1) * 128)
                pp = ps2()
                for half in range(2):
                    jsl = slice(half * 384, (half + 1) * 384)
                    for k in range(KD):
                        nc.tensor.matmul(pp[:, half, :384],
                                         lhsT=xTb[:, k, csl],
                                         rhs=wv_sb[:, k, jsl],
                                         start=(k == 0), stop=(k == KD - 1))
                nc.vector.tensor_add(
                    v_sb[:, c, :].rearrange("p (h j) -> p h j", h=2),
                    pp[:, :, :384],
                    bv_sb[:, :].rearrange("p (h j) -> p h j", h=2))

            # ---- Q,K + attention, per head-pair ----
            for pr in range(PAIRS):
                # q,k GEMMs for this pair into one [128, 2, T] tile
                qk_t = qkpool.tile([128, 2, T], BF16, tag="qk")
                for mi, m in enumerate((pr, PAIRS + pr)):
                    w_sb = wpool6.tile([128, D], BF16, tag="w6")
                    nc.sync.dma_start(w_sb[:], wqk[l, m])
                    pp = ps2()
                    for tch in range(NT):
                        tsl = slice(tch * TCH, (tch + 1) * TCH)
                        for k in range(KD):
                            nc.tensor.matmul(pp[:, tch, :],
                                             lhsT=w_sb[:, k * 128:(k + 1) * 128],
                                             rhs=xTb[:, k, tsl],
                                             start=(k == 0), stop=(k == KD - 1))
                    nc.scalar.activation(qk_t[:, mi, :], pp[:], AF.Identity,
                                         bias=bqk_sb[:, m:m + 1])
                # QK logits + exp, all (s, e, chunk-pair) — row-tiled matmuls
                expP_all = {}
                for s in range(B_LOC):
                    s0 = s * S
                    for e in range(2):
                        expP_all[(s, e)] = attnp.tile([128, 4, S], BF16,
                                                      tag="expP", name="expP")
                for s in range(B_LOC):
                    s0 = s * S
                    for cp in range(2):          # chunk pairs (st=2cp, 2cp+1)
                        pse = {}
                        for e in range(2):
                            po = 64 * e
                            pp = ps2()
                            for sti in range(2):
                                st = 2 * cp + sti
                                ksl = slice(s0 + st * 128, s0 + (st + 1) * 128)
                                nc.tensor.matmul(
                                    pp[:, sti, :],
                                    lhsT=qk_t[po:po + 64, 1, ksl],
                                    rhs=qk_t[po:po + 64, 0, s0:s0 + S],
                                    start=True, stop=True,
                                    tile_position=(po, 0))
                            pse[e] = pp
                        for e in range(2):
                            nc.scalar.activation(
                                expP_all[(s, e)][:, 2 * cp:2 * cp + 2, :],
                                pse[e][:], AF.Exp, scale=SCALE)
                if dbg and l == 0 and pr == PAIRS - 1:
                    nc.sync.dma_start(dbg['expP'], expP_all[(0, 0)][:])
                # attn@V + denominator — col-tiled over the two heads
                for s in range(B_LOC):
                    s0 = s * S
                    expP0, expP1 = expP_all[(s, 0)], expP_all[(s, 1)]
                    psu0, psu1 = ps1(), ps1()
                    psd0, psd1 = ps1(), ps1()
                    for st in range(4):
                        c = s * 4 + st
                        st_, sp_ = (st == 0), (st == 3)
                        nc.tensor.matmul(
                            psu0[0:64, :],
                            lhsT=v_sb[:, c, pr * 128:pr * 128 + 64],
                            rhs=expP0[:, st, :], start=st_, stop=sp_,
                            tile_position=(0, 0))
                        nc.tensor.matmul(
                            psu1[64:128, :],
                            lhsT=v_sb[:, c, pr * 128 + 64:(pr + 1) * 128],
                            rhs=expP1[:, st, :], start=st_, stop=sp_,
                            tile_position=(0, 64))
                        nc.tensor.matmul(
                            psd0[0:64, :], lhsT=onesb_sb[:, 0:64],
                            rhs=expP0[:, st, :], start=st_, stop=sp_,
                            tile_position=(0, 0))
                        nc.tensor.matmul(
                            psd1[64:128, :], lhsT=onesb_sb[:, 64:128],
                            rhs=expP1[:, st, :], start=st_, stop=sp_,
                            tile_position=(0, 64))
                    u_sb = recp.tile([128, S], F32, tag="u")
                    rec0 = recp.tile([128, S], F32, tag="rec0")
                    rec1 = recp.tile([128, S], F32, tag="rec1")
                    nc.scalar.activation(u_sb[0:64, :], psu0[0:64, :], AF.Identity)
                    nc.scalar.activation(u_sb[64:128, :], psu1[64:128, :],
                                         AF.Identity)
                    nc.vector.reciprocal_approx_fast(rec0[:], psd0[:])
                    nc.vector.reciprocal_approx_fast(rec1[:], psd1[:])
                    if dbg and l == 0 and pr == PAIRS - 1 and s == 0:
                        nc.sync.dma_start(dbg['psu0'], u_sb[:])
                        nc.sync.dma_start(dbg['rec'], rec0[:])
                    nc.vector.tensor_mul(aT[0:64, pr, s0:s0 + S],
                                         u_sb[0:64, :], rec0[0:64, :])
                    nc.vector.tensor_mul(aT[64:128, pr, s0:s0 + S],
                                         u_sb[64:128, :], rec1[64:128, :])

            if dbg and l == 0:
                nc.sync.dma_start(dbg['v'], v_sb[:])
                nc.sync.dma_start(dbg['aT'], aT[:])
                nc.sync.dma_start(dbg['qk'], qk_t[:])
            # ---- proj + residual into xT (tch-major: LN1(tch0) overlaps
            # proj(tch1) on PE) ----
            nc.scalar.activation(scratch[:, 2:3], aT[:, PAIRS - 1, T - 1:T],
                                 AF.Sqrt)
            pw_tiles = []
            for m in range(KD):
                w_sb = wpool6.tile([128, D], BF16, tag="w6")
                nc.sync.dma_start(w_sb[:], wproj[l, m])
                pw_tiles.append(w_sb)
            for tch in range(NT):
                tsl = slice(tch * TCH, (tch + 1) * TCH)
                for m in range(KD):
                    ps = ps1()
                    for k in range(KD):
                        nc.tensor.matmul(ps[:],
                                         lhsT=pw_tiles[m][:, k * 128:(k + 1) * 128],
                                         rhs=aT[:, k, tsl],
                                         start=(k == 0), stop=(k == KD - 1))
                    nc.vector.scalar_tensor_tensor(
                        xT[:, m, tsl], in0=ps[:], scalar=bp_sb[:, m:m + 1],
                        in1=xT[:, m, tsl], op0=OP.add, op1=OP.add)

            layer_norm_T(l1w_sb, l1b_sb, lnp_pool, True)
            if dbg and l == 0:
                nc.sync.dma_start(dbg['x1'], xT[:])
                nc.sync.dma_start(dbg['xb1'], xTb[:])
            # prefetch the gelu ACT table set while MLP1 matmuls run (the
            # xTb dep pins this after LN1's exp-family ops)
            nc.scalar.activation(scratch[:, 0:1], xTb[:, 0, T - 1:T], AF.Gelu)

            # ---- MLP1 (tch-major: LN1(tch1) overlaps MLP1(tch0)) ----
            for tch in range(NT):
                tsl = slice(tch * TCH, (tch + 1) * TCH)
                for m in range(FM):
                    w_sb = wpool6.tile([128, D], BF16, tag="w6")
                    nc.sync.dma_start(w_sb[:], w1[l, m])
                    ps = ps1()
                    for k in range(KD):
                        nc.tensor.matmul(ps[:],
                                         lhsT=w_sb[:, k * 128:(k + 1) * 128],
                                         rhs=xTb[:, k, tsl],
                                         start=(k == 0), stop=(k == KD - 1))
                    nc.scalar.activation(h[:, m, tsl], ps[:], AF.Gelu,
                                         bias=b1_sb[:, m:m + 1])
            # prefetch the sqrt table set while MLP2 matmuls run (dep on
            # the last gelu's output pins this after all gelus)
            nc.scalar.activation(scratch[:, 1:2], h[:, FM - 1, T - 1:T], AF.Sqrt)
            for tch in range(NT):
                tsl = slice(tch * TCH, (tch + 1) * TCH)
                for m in range(KD):
                    w2_sb = wpool24.tile([128, F], BF16, tag="w24")
                    nc.sync.dma_start(w2_sb[:], w2[l, m])
                    ps = ps1()
                    for k in range(FM):
                        nc.tensor.matmul(ps[:],
                                         lhsT=w2_sb[:, k * 128:(k + 1) * 128],
                                         rhs=h[:, k, tsl],
                                         start=(k == 0), stop=(k == FM - 1))
                    nc.vector.scalar_tensor_tensor(
                        xT[:, m, tsl], in0=ps[:], scalar=b2_sb[:, m:m + 1],
                        in1=xT[:, m, tsl], op0=OP.add, op1=OP.add)

            layer_norm_T(l2w_sb, l2b_sb, lnp_pool, l < n_layers - 1)
            # prefetch the exp table set for the next layer's attention
            nc.scalar.activation(scratch[:, 3:4], xTb[:, 0, T - 1:T], AF.Exp)
            if dbg and l == 0:
                nc.sync.dma_start(dbg['h'], h[:])
                nc.sync.dma_start(dbg['xbf'], xTb[:])
        lctx.close()

        # ---------------- Pooler ----------------
        with tc.tile_pool(name="poolp", bufs=1) as poolp:
            bpl_sb = poolp.tile([128, KD], F32)
            nc.sync.dma_start(bpl_sb[:], bpool.rearrange("(m p) -> p m", p=128))
            pool_sb = poolp.tile([128, KD, B_LOC], F32R)
            for m in range(KD):
                w_sb = poolp.tile([128, D], BF16, tag="wp", name="w_sb")
                nc.sync.dma_start(w_sb[:], wpool[m])
                ps = ps1()
                for k in range(KD):
                    first_tok = xTb[:, k, :].rearrange("p (b s) -> p b s", s=S)
                    nc.tensor.matmul(ps[:, :B_LOC],
                                     lhsT=w_sb[:, k * 128:(k + 1) * 128],
                                     rhs=first_tok[:, :, 0:1],
                                     start=(k == 0), stop=(k == KD - 1))
                nc.scalar.activation(pool_sb[:, m, :], ps[:, :B_LOC], AF.Tanh,
                                     bias=bpl_sb[:, m:m + 1])
            out_sb = poolp.tile([128, D], F32)
            for k in range(KD):
                pst = psum.tile([128, 128], F32R, tag="mm1", name="pst")
                nc.tensor.transpose(pst[:B_LOC, :], pool_sb[:, k, :], ident_sb[:])
                nc.vector.tensor_copy(out_sb[:B_LOC, k * 128:(k + 1) * 128],
                                      pst[:B_LOC, :])
            nc.sync.dma_start(out, out_sb[:B_LOC, :])


def _prep_host(inputs, n_layers=L):
    f32 = lambda a: np.asarray(a, dtype=np.float32)
    tokens = np.asarray(inputs["tokens"]).astype(np.int32)          # [16, 512]
    possent = f32(inputs["pos_emb"])[0] + f32(inputs["sent_emb"])[0, 0][None, :]

    nl = max(1, n_layers)
    # Fold layer-norm bias AND weight into the downstream GEMM (the device's
    # bf16 shadow xTb is (x-mu)*rstd only):
    #   W_eff[l]  = W[l] @ diag(prev_ln_w)
    #   b_eff[l]  = b[l] + W[l] @ prev_ln_b
    qkv_b = f32(inputs["qkv_b"]).copy()
    mlp_b1 = f32(inputs["mlp_b1"]).copy()
    pool_b = f32(inputs["pool_b"]).copy()
    emb_ln_w = f32(inputs["emb_ln_w"])
    emb_ln_b = f32(inputs["emb_ln_b"])
    ln1_w = f32(inputs["ln1_w"])
    ln1_b = f32(inputs["ln1_b"])
    ln2_w = f32(inputs["ln2_w"])
    ln2_b = f32(inputs["ln2_b"])

    qkv_w_eff = []
    mlp_w1_eff = []
    for l in range(nl):
        prev_w = emb_ln_w if l == 0 else ln2_w[l - 1]
        prev_b = emb_ln_b if l == 0 else ln2_b[l - 1]
        qw = f32(inputs["qkv_w"][l])
        qkv_b[l] = qkv_b[l] + qw @ prev_b
        qkv_w_eff.append(qw * prev_w[None, :])
        w1l = f32(inputs["mlp_w1"][l])
        mlp_b1[l] = mlp_b1[l] + w1l @ ln1_b[l]
        mlp_w1_eff.append(w1l * ln1_w[l][None, :])
    last_w = ln2_w[nl - 1] if n_layers >= 1 else emb_ln_w
    last_b = ln2_b[nl - 1] if n_layers >= 1 else emb_ln_b
    pw = f32(inputs["pool_w"])
    pool_b = pool_b + pw @ last_b
    pool_w_eff = pw * last_w[None, :]

    # Q,K weight tiles [nl, 12, 128, D]; V as wvT [nl, 128, KD, D]
    wqk = np.stack([_host_tile_weight(qkv_w_eff[l][:2 * D]) for l in range(nl)])
    wvT = np.stack([
        np.ascontiguousarray(
            qkv_w_eff[l][2 * D:].T.reshape(KD, 128, D).transpose(1, 0, 2)
        ).astype(ml_dtypes.bfloat16)
        for l in range(nl)])

    common = {
        "tok_emb": f32(inputs["tok_emb"]),
        "possent": possent.astype(np.float32),
        "embw": emb_ln_w,
        "embb": emb_ln_b,
        "wqk": wqk,
        "bqk": qkv_b[:nl, :2 * D],
        "wvT": wvT,
        "bv": qkv_b[:nl, 2 * D:],
        "wproj": np.stack([_host_tile_weight(f32(inputs["proj_w"][l]))
                           for l in range(nl)]),
        "bproj": f32(inputs["proj_b"])[:nl],
        "w1": np.stack([_host_tile_weight(mlp_w1_eff[l]) for l in range(nl)]),
        "b1": mlp_b1[:nl],
        "w2": np.stack([_host_tile_weight(f32(inputs["mlp_w2"][l]))
                        for l in range(nl)]),
        "b2": f32(inputs["mlp_b2"])[:nl],
        "ln1w": ln1_w[:nl],
        "ln1b": ln1_b[:nl],
        "ln2w": ln2_w[:nl],
        "ln2b": ln2_b[:nl],
        "wpool": _host_tile_weight(pool_w_eff),
        "bpool": pool_b,
        "ident": np.eye(128, dtype=np.float32),
        "ones": np.ones((128, 128), dtype=np.float32),
        "ones_bf": np.ones((128, 128), dtype=ml_dtypes.bfloat16),
    }
    in_maps = []
    for c in range(NCORES):
        m = dict(common)
        m["tokens"] = np.ascontiguousarray(
            tokens[c * B_LOC:(c + 1) * B_LOC].reshape(-1))
        in_maps.append(m)
    return in_maps


def kernel(**inputs) -> np.ndarray:
    global _CACHED_NC
    if _CACHED_NC is None:
        _CACHED_NC = build_nc(L)
    in_maps = _prep_host(inputs, L)
    res = run_bass_kernel_spmd(_CACHED_NC, in_maps,
                               core_ids=list(range(NCORES)), trace=False)
    return np.concatenate([res.results[c]["out"] for c in range(NCORES)], axis=0)


# revision 21
# speedup vs baseline: 1.1944x; 1.1944x over previous
"""BERT-base forward (B=16, S=512, D=768, H=12, L=12) on 8 Trainium2 NeuronCores.

Sharding: data-parallel over batch — each core runs 2 sequences (1024 tokens)
with a full replica of the weights. No collectives.

Device layout: "transposed activations" — activations live as x^T [D, tokens]
(features on SBUF partitions, tokens on the free dim), so every linear layer is
out^T = W^T.T @ x^T with the (host-pre-tiled) weight as the stationary operand
and 512-token chunks as the moving operand.

v2 structure (vs the v1 baseline):
- V is computed by a "flipped" GEMM (x^T chunks stationary, V-weight moving),
  producing V directly in [token, head_dim] orientation — no DMA transposes.
- QK logits use PE row-tiling (two 64-contraction head matmuls concurrent via
  tile_position (0,0)/(64,0)); attn@V and the ones-denominator matmuls use PE
  col-tiling ((0,0)/(0,64)) so both heads of a pair run concurrently.
- LayerNorm: ln scale is folded into the downstream GEMM weights on the host,
  so the bf16 GEMM shadow is just (x-mean)*rstd; rstd = exp(-0.5*ln(var+eps))
  keeps all transcendentals in the natural_log_exp ACT table set (shared with
  attention's exp) — only gelu forces table swaps, prefetched via dummy ops.
- Attention softmax normalization reads PSUM directly on DVE (reciprocal +
  multiply), no ACT identity copies.
"""
import sys
sys.path.insert(0, '/opt/trn_rl_repo')

import numpy as np
import ml_dtypes
import concourse.bass as bass
import concourse.tile as tile
from concourse import bacc, mybir
from concourse.bass_utils import run_bass_kernel_spmd

# Model shapes (hardcoded)
V = 30522
S = 512
D = 768
H = 12
L = 12
F = 3072
B = 16
HD = 64
EPS = 1e-12
SCALE = HD ** (-0.5)

NCORES = 8
B_LOC = B // NCORES          # 2 sequences per core
T = B_LOC * S                # 1024 tokens per core
KD = D // 128                # 6 k-tiles over D
QKM = 2 * D // 128           # 12 m-tiles for q,k
FM = F // 128                # 24 m-tiles over mlp hidden
PAIRS = H // 2               # 6 head-pairs
TCH = 512                    # token chunk for all GEMMs
NT = T // TCH                # 2
NC = T // 128                # 8 128-token chunks

F32 = mybir.dt.float32
F32R = mybir.dt.float32r
BF16 = mybir.dt.bfloat16
I32 = mybir.dt.int32
AF = mybir.ActivationFunctionType
OP = mybir.AluOpType

_CACHED_NC = None
DEBUG = False


def _host_tile_weight(w_t):
    """w_t: [dout, din] torch-Linear weight. Returns [m_tiles, 128, din] where
    slice [m] is (w_t.T)[:, m*128:(m+1)*128] laid out partition-major."""
    dout, din = w_t.shape
    m_tiles, k_tiles = dout // 128, din // 128
    a = np.ascontiguousarray(w_t.T)                      # [din, dout]
    a = a.reshape(k_tiles, 128, m_tiles, 128)            # [k, p, m, w]
    a = np.ascontiguousarray(a.transpose(2, 1, 0, 3))    # [m, p, k, w]
    return a.reshape(m_tiles, 128, din).astype(ml_dtypes.bfloat16)


def build_nc(n_layers=L):
    nc = bacc.Bacc("TRN2", target_bir_lowering=False, debug=False)

    def din(name, shape, dt=BF16):
        return nc.dram_tensor(name, shape, dt, kind="ExternalInput").ap()

    nl = max(1, n_layers)
    tokens = din("tokens", [T], I32)
    tok_emb = din("tok_emb", [V, D], F32R)
    possent = din("possent", [S, D], F32)
    embw = din("embw", [D], F32)
    embb = din("embb", [D], F32)
    wqk = din("wqk", [nl, QKM, 128, D])
    bqk = din("bqk", [nl, 2 * D], F32)
    wvT = din("wvT", [nl, 128, KD, D])
    bv = din("bv", [nl, D], F32)
    wproj = din("wproj", [nl, KD, 128, D])
    bproj = din("bproj", [nl, D], F32)
    w1 = din("w1", [nl, FM, 128, D])
    b1 = din("b1", [nl, F], F32)
    w2 = din("w2", [nl, KD, 128, F])
    b2 = din("b2", [nl, D], F32)
    ln1w = din("ln1w", [nl, D], F32)
    ln1b = din("ln1b", [nl, D], F32)
    ln2w = din("ln2w", [nl, D], F32)
    ln2b = din("ln2b", [nl, D], F32)
    wpool = din("wpool", [KD, 128, D], BF16)
    bpool = din("bpool", [D], F32)
    ident = din("ident", [128, 128], F32R)
    ones = din("ones", [128, 128], F32R)
    ones_bf = din("ones_bf", [128, 128], BF16)
    out = nc.dram_tensor("out", [B_LOC, D], F32, kind="ExternalOutput").ap()
    dbg = {}
    if DEBUG:
        dbg['v'] = nc.dram_tensor("dbg_v", [128, NC, D], BF16, kind="ExternalOutput").ap()
        dbg['aT'] = nc.dram_tensor("dbg_aT", [128, KD, T], BF16, kind="ExternalOutput").ap()
        dbg['x1'] = nc.dram_tensor("dbg_x1", [128, KD, T], F32R, kind="ExternalOutput").ap()
        dbg['xb1'] = nc.dram_tensor("dbg_xb1", [128, KD, T], BF16, kind="ExternalOutput").ap()
        dbg['h'] = nc.dram_tensor("dbg_h", [128, FM, T], BF16, kind="ExternalOutput").ap()
        dbg['xbf'] = nc.dram_tensor("dbg_xbf", [128, KD, T], BF16, kind="ExternalOutput").ap()
        dbg['qk'] = nc.dram_tensor("dbg_qk", [128, 2, T], BF16, kind="ExternalOutput").ap()
        dbg['expP'] = nc.dram_tensor("dbg_expP", [128, 4, S], BF16, kind="ExternalOutput").ap()
        dbg['psu0'] = nc.dram_tensor("dbg_psu0", [128, S], F32, kind="ExternalOutput").ap()
        dbg['psd0'] = nc.dram_tensor("dbg_psd0", [128, S], F32, kind="ExternalOutput").ap()
        dbg['rec'] = nc.dram_tensor("dbg_rec", [128, S], F32, kind="ExternalOutput").ap()

    with tile.TileContext(nc) as tc:
        _build_body(nc, tc, n_layers, tokens, tok_emb, possent, embw, embb,
                    wqk, bqk, wvT, bv, wproj, bproj, w1, b1, w2, b2,
                    ln1w, ln1b, ln2w, ln2b, wpool, bpool, ident, ones, ones_bf,
                    out, dbg)
    nc.compile()
    return nc


def _build_body(nc, tc, n_layers, tokens, tok_emb, possent, embw, embb,
                wqk, bqk, wvT, bv, wproj, bproj, w1, b1, w2, b2,
                ln1w, ln1b, ln2w, ln2b, wpool, bpool, ident, ones, ones_bf,
                out, dbg={}):
    from contextlib import ExitStack
    ctx = ExitStack()
    with ctx:
        consts = ctx.enter_context(tc.tile_pool(name="consts", bufs=1))
        xpool = ctx.enter_context(tc.tile_pool(name="xpool", bufs=1))
        qkpool = ctx.enter_context(tc.tile_pool(name="qkpool", bufs=3))
        wpool6 = ctx.enter_context(tc.tile_pool(name="wpool6", bufs=8))
        psum = ctx.enter_context(tc.tile_pool(name="psum", bufs=4, space="PSUM"))
        psum2 = ctx.enter_context(tc.tile_pool(name="psum2", bufs=2, space="PSUM"))

        ident_sb = consts.tile([128, 128], F32R)
        nc.sync.dma_start(ident_sb[:], ident)
        ones_sb = consts.tile([128, 128], F32R)
        nc.sync.dma_start(ones_sb[:], ones)
        onesb_sb = consts.tile([128, 128], BF16)
        nc.sync.dma_start(onesb_sb[:], ones_bf)
        eps_sb = consts.tile([128, 1], F32)
        nc.vector.memset(eps_sb[:], EPS)
        scratch = consts.tile([128, 8], F32)

        # residual stream x^T (fp32) + bf16 shadow (= (x-mu)*rstd, no w/b)
        xT = xpool.tile([128, KD, T], F32R)
        xTb = xpool.tile([128, KD, T], BF16)
        # attention output a^T (bf16: proj GEMM input)
        aT = xpool.tile([128, KD, T], BF16)
        # V in [token, dout] orientation: v_sb[p, c, j] = v[token c*128+p, j]
        v_sb = xpool.tile([128, NC, D], BF16)

        def ps1():
            return psum.tile([128, TCH], F32, tag="mm1", name="ps1")

        def ps2():
            return psum2.tile([128, 2, TCH], F32, tag="mm2", name="ps2")

        # ---------------- Embedding ----------------
        with tc.tile_pool(name="embp", bufs=2) as embp:
            for tt in range(T // 128):
                idx_sb = embp.tile([128, 1], I32, tag="idx")
                nc.sync.dma_start(idx_sb[:], tokens[tt * 128:(tt + 1) * 128, None])
                g_sb = embp.tile([128, D], F32R, tag="g")
                nc.gpsimd.indirect_dma_start(
                    out=g_sb[:], out_offset=None, in_=tok_emb,
                    in_offset=bass.IndirectOffsetOnAxis(ap=idx_sb[:, :1], axis=0))
                p_sb = embp.tile([128, D], F32, tag="p")
                prow = (tt * 128) % S
                nc.sync.dma_start(p_sb[:], possent[prow:prow + 128, :])
                nc.vector.tensor_add(g_sb[:], g_sb[:], p_sb[:])
                # LayerNorm over free dim (d): bn_stats in 2 subgroups of 384
                st_sb = embp.tile([128, 2, 6], F32, tag="st")
                gv = g_sb[:].rearrange("p (a b) -> p a b", a=2)
                for a in range(2):
                    nc.vector.bn_stats(st_sb[:, a, :], gv[:, a, :])
                mv = embp.tile([128, 2], F32, tag="mv")
                nc.vector.bn_aggr(mv[:], st_sb[:])
                sd = embp.tile([128, 1], F32, tag="sd")
                nc.scalar.activation(sd[:], mv[:, 1:2], AF.Sqrt, bias=eps_sb[:])
                nc.vector.reciprocal_approx_fast(sd[:], sd[:])
                nc.vector.tensor_scalar(g_sb[:], g_sb[:], mv[:, 0:1], sd[:],
                                        op0=OP.subtract, op1=OP.mult)
                # transpose (x-mu)*rstd into xT; w/b applied after in
                # transposed space (w is folded into l=0 GEMM weights for the
                # shadow; xT carries the full LN output for the residual)
                for k in range(KD):
                    pst = psum.tile([128, 128], F32R, tag="mm1", name="pst")
                    nc.tensor.transpose(pst[:], g_sb[:, k * 128:(k + 1) * 128],
                                        ident_sb[:])
                    nc.vector.tensor_copy(xT[:, k, tt * 128:(tt + 1) * 128], pst[:])
            nc.vector.tensor_copy(xTb[:], xT[:])
            embwk_sb = embp.tile([128, KD], F32, tag="embwk")
            nc.sync.dma_start(embwk_sb[:], embw.rearrange("(k p) -> p k", p=128))
            embbk_sb = embp.tile([128, KD], F32, tag="embbk")
            nc.sync.dma_start(embbk_sb[:], embb.rearrange("(k p) -> p k", p=128))
            for k in range(KD):
                nc.gpsimd.tensor_scalar(xT[:, k, :], xT[:, k, :],
                                        embwk_sb[:, k:k + 1], embbk_sb[:, k:k + 1],
                                        op0=OP.mult, op1=OP.add)

        # ---------------- Layer norm (transposed layout) ----------------
        def layer_norm_T(lw_sb, lb_sb, lnp, finalize):
            # stats via all-ones matmuls on the fp32 residual. Shadow xTb gets
            # (x-mu)*rstd (ln w folded into downstream weights, bias folded
            # into downstream GEMM bias on the host). xT gets the full LN
            # output off the critical path.
            for tch in range(NT):
                tsl = slice(tch * TCH, (tch + 1) * TCH)
                xs = xT[:, :, tsl]
                ps_s = ps1()
                for k in range(KD):
                    nc.tensor.matmul(ps_s[:], lhsT=ones_sb[:], rhs=xT[:, k, tsl],
                                     start=(k == 0), stop=(k == KD - 1))
                ps_q = ps1()
                for k in range(KD):
                    sq = lnp.tile([128, TCH], F32R, tag="sq")
                    if k < 3:
                        nc.vector.tensor_mul(sq[:], xT[:, k, tsl], xT[:, k, tsl])
                    else:
                        nc.gpsimd.tensor_mul(sq[:], xT[:, k, tsl], xT[:, k, tsl])
                    nc.tensor.matmul(ps_q[:], lhsT=ones_sb[:], rhs=sq[:],
                                     start=(k == 0), stop=(k == KD - 1))
                mean = lnp.tile([128, TCH], F32, tag="mean")
                nc.vector.tensor_scalar_mul(mean[:], ps_s[:], 1.0 / D)
                var = lnp.tile([128, TCH], F32, tag="var")
                nc.vector.tensor_mul(var[:], mean[:], mean[:])
                nc.vector.scalar_tensor_tensor(var[:], in0=ps_q[:], scalar=1.0 / D,
                                               in1=var[:], op0=OP.mult,
                                               op1=OP.subtract)
                # rstd = exp(-0.5*ln(var+eps)) — stays in the ln/exp ACT set
                r = lnp.tile([128, TCH], F32, tag="r")
                nc.scalar.activation(r[:], var[:], AF.Sqrt, bias=eps_sb[:])
                nc.vector.reciprocal_approx_fast(r[:], r[:])
                mean_bc4 = mean[:, None, :].to_broadcast([128, 4, TCH])
                mean_bc2 = mean[:, None, :].to_broadcast([128, 2, TCH])
                nc.vector.tensor_sub(xs[:, 0:4, :], xs[:, 0:4, :], mean_bc4)
                nc.gpsimd.tensor_sub(xs[:, 4:6, :], xs[:, 4:6, :], mean_bc2)
                r_bc4 = r[:, None, :].to_broadcast([128, 4, TCH])
                r_bc2 = r[:, None, :].to_broadcast([128, 2, TCH])
                nc.vector.tensor_mul(xs[:, 0:4, :], xs[:, 0:4, :], r_bc4)
                nc.gpsimd.tensor_mul(xs[:, 4:6, :], xs[:, 4:6, :], r_bc2)
                # bf16 shadow for the GEMMs (critical path ends here)
                nc.vector.tensor_copy(xTb[:, 0:3, tsl], xs[:, 0:3, :])
                nc.vector.tensor_copy(xTb[:, 3:6, tsl], xs[:, 3:6, :])
                if finalize:
                    # xT = shadow*w + b (full LN output for the residual)
                    for k in range(KD):
                        nc.gpsimd.tensor_scalar(xs[:, k, :], xs[:, k, :],
                                                lw_sb[:, k:k + 1],
                                                lb_sb[:, k:k + 1],
                                                op0=OP.mult, op1=OP.add)

        # ---------------- Layers ----------------
        lctx = ExitStack()
        hpool = lctx.enter_context(tc.tile_pool(name="hpool", bufs=1))
        wpool24 = lctx.enter_context(tc.tile_pool(name="wpool24", bufs=2))
        wvpool = lctx.enter_context(tc.tile_pool(name="wvpool", bufs=1))
        biasp = lctx.enter_context(tc.tile_pool(name="biasp", bufs=1))
        attnp = lctx.enter_context(tc.tile_pool(name="attnp", bufs=4))
        recp = lctx.enter_context(tc.tile_pool(name="recp", bufs=2))
        lnp_pool = lctx.enter_context(tc.tile_pool(name="lnp", bufs=2))
        h = hpool.tile([128, FM, T], BF16, tag="h")
        for l in range(n_layers):
            # per-layer bias/ln tiles
            bqk_sb = biasp.tile([128, QKM], F32, tag="bqk")
            nc.sync.dma_start(bqk_sb[:], bqk[l].rearrange("(m p) -> p m", p=128))
            bv_sb = biasp.tile([128, D], F32, tag="bv")
            nc.sync.dma_start(bv_sb[:], bv[l][None, :].to_broadcast([128, D]))
            bp_sb = biasp.tile([128, KD], F32, tag="bp")
            nc.sync.dma_start(bp_sb[:], bproj[l].rearrange("(m p) -> p m", p=128))
            b1_sb = biasp.tile([128, FM], F32, tag="b1")
            nc.sync.dma_start(b1_sb[:], b1[l].rearrange("(m p) -> p m", p=128))
            b2_sb = biasp.tile([128, KD], F32, tag="b2")
            nc.sync.dma_start(b2_sb[:], b2[l].rearrange("(m p) -> p m", p=128))
            l1w_sb = biasp.tile([128, KD], F32, tag="l1w")
            nc.sync.dma_start(l1w_sb[:], ln1w[l].rearrange("(k p) -> p k", p=128))
            l1b_sb = biasp.tile([128, KD], F32, tag="l1b")
            nc.sync.dma_start(l1b_sb[:], ln1b[l].rearrange("(k p) -> p k", p=128))
            l2w_sb = biasp.tile([128, KD], F32, tag="l2w")
            nc.sync.dma_start(l2w_sb[:], ln2w[l].rearrange("(k p) -> p k", p=128))
            l2b_sb = biasp.tile([128, KD], F32, tag="l2b")
            nc.sync.dma_start(l2b_sb[:], ln2b[l].rearrange("(k p) -> p k", p=128))

            # ---- V via flipped GEMM: v[tok, j] directly (no transposes) ----
            wv_sb = wvpool.tile([128, KD, D], BF16, tag="wv")
            nc.sync.dma_start(wv_sb[:], wvT[l])
            for c in range(NC):
                csl = slice(c * 128, (c + 1) * 128)
                pp = ps2()
                for half in range(2):
                    jsl = slice(half * 384, (half + 1) * 384)
                    for k in range(KD):
                        nc.tensor.matmul(pp[:, half, :384],
                                         lhsT=xTb[:, k, csl],
                                         rhs=wv_sb[:, k, jsl],
                                         start=(k == 0), stop=(k == KD - 1))
                nc.vector.tensor_add(
                    v_sb[:, c, :].rearrange("p (h j) -> p h j", h=2),
                    pp[:, :, :384],
                    bv_sb[:, :].rearrange("p (h j) -> p h j", h=2))

            # ---- Q,K + attention, per head-pair ----
            for pr in range(PAIRS):
                # q,k GEMMs for this pair into one [128, 2, T] tile
                qk_t = qkpool.tile([128, 2, T], BF16, tag="qk")
                for mi, m in enumerate((pr, PAIRS + pr)):
                    w_sb = wpool6.tile([128, D], BF16, tag="w6")
                    nc.sync.dma_start(w_sb[:], wqk[l, m])
                    pp = ps2()
                    for tch in range(NT):
                        tsl = slice(tch * TCH, (tch + 1) * TCH)
                        for k in range(KD):
                            nc.tensor.matmul(pp[:, tch, :],
                                             lhsT=w_sb[:, k * 128:(k + 1) * 128],
                                             rhs=xTb[:, k, tsl],
                                             start=(k == 0), stop=(k == KD - 1))
                    nc.scalar.activation(qk_t[:, mi, :], pp[:], AF.Identity,
                                         bias=bqk_sb[:, m:m + 1])
                # QK logits + exp, all (s, e, chunk-pair) — row-tiled matmuls
                expP_all = {}
                for s in range(B_LOC):
                    s0 = s * S
                    for e in range(2):
                        expP_all[(s, e)] = attnp.tile([128, 4, S], BF16,
                                                      tag="expP", name="expP")
                for s in range(B_LOC):
                    s0 = s * S
                    for cp in range(2):          # chunk pairs (st=2cp, 2cp+1)
                        pse = {}
                        for e in range(2):
                            po = 64 * e
                            pp = ps2()
                            for sti in range(2):
                                st = 2 * cp + sti
                                ksl = slice(s0 + st * 128, s0 + (st + 1) * 128)
                                nc.tensor.matmul(
                                    pp[:, sti, :],
                                    lhsT=qk_t[po:po + 64, 1, ksl],
                                    rhs=qk_t[po:po + 64, 0, s0:s0 + S],
                                    start=True, stop=True,
                                    tile_position=(po, 0))
                            pse[e] = pp
                        for e in range(2):
                            nc.scalar.activation(
                                expP_all[(s, e)][:, 2 * cp:2 * cp + 2, :],
                                pse[e][:], AF.Exp, scale=SCALE)
                if dbg and l == 0 and pr == PAIRS - 1:
                    nc.sync.dma_start(dbg['expP'], expP_all[(0, 0)][:])
                # attn@V + denominator — col-tiled over the two heads
                for s in range(B_LOC):
                    s0 = s * S
                    expP0, expP1 = expP_all[(s, 0)], expP_all[(s, 1)]
                    psu0, psu1 = ps1(), ps1()
                    psd0, psd1 = ps1(), ps1()
                    for st in range(4):
                        c = s * 4 + st
                        st_, sp_ = (st == 0), (st == 3)
                        nc.tensor.matmul(
                            psu0[0:64, :],
                            lhsT=v_sb[:, c, pr * 128:pr * 128 + 64],
                            rhs=expP0[:, st, :], start=st_, stop=sp_,
                            tile_position=(0, 0))
                        nc.tensor.matmul(
                            psu1[64:128, :],
                            lhsT=v_sb[:, c, pr * 128 + 64:(pr + 1) * 128],
                            rhs=expP1[:, st, :], start=st_, stop=sp_,
                            tile_position=(0, 64))
                        nc.tensor.matmul(
                            psd0[0:64, :], lhsT=onesb_sb[:, 0:64],
                            rhs=expP0[:, st, :], start=st_, stop=sp_,
                            tile_position=(0, 0))
                        nc.tensor.matmul(
                            psd1[64:128, :], lhsT=onesb_sb[:, 64:128],
                            rhs=expP1[:, st, :], start=st_, stop=sp_,
                            tile_position=(0, 64))
                    u_sb = recp.tile([128, S], F32, tag="u")
                    rec0 = recp.tile([128, S], F32, tag="rec0")
                    rec1 = recp.tile([128, S], F32, tag="rec1")
                    nc.scalar.activation(u_sb[0:64, :], psu0[0:64, :], AF.Identity)
                    nc.scalar.activation(u_sb[64:128, :], psu1[64:128, :],
                                         AF.Identity)
                    nc.vector.reciprocal_approx_fast(rec0[:], psd0[:])
                    nc.vector.reciprocal_approx_fast(rec1[:], psd1[:])
                    if dbg and l == 0 and pr == PAIRS - 1 and s == 0:
                        nc.sync.dma_start(dbg['psu0'], u_sb[:])
                        nc.sync.dma_start(dbg['rec'], rec0[:])
                    nc.vector.tensor_mul(aT[0:64, pr, s0:s0 + S],
                                         u_sb[0:64, :], rec0[0:64, :])
                    nc.vector.tensor_mul(aT[64:128, pr, s0:s0 + S],
                                         u_sb[64:128, :], rec1[64:128, :])

            if dbg and l == 0:
                nc.sync.dma_start(dbg['v'], v_sb[:])
                nc.sync.dma_start(dbg['aT'], aT[:])
                nc.sync.dma_start(dbg['qk'], qk_t[:])
            # ---- proj + residual into xT (tch-major: LN1(tch0) overlaps
            # proj(tch1) on PE) ----
            nc.scalar.activation(scratch[:, 2:3], aT[:, PAIRS - 1, T - 1:T],
                                 AF.Sqrt)
            pw_tiles = []
            for m in range(KD):
                w_sb = wpool6.tile([128, D], BF16, tag="w6")
                nc.sync.dma_start(w_sb[:], wproj[l, m])
                pw_tiles.append(w_sb)
            for tch in range(NT):
                tsl = slice(tch * TCH, (tch + 1) * TCH)
                for m in range(KD):
                    ps = ps1()
                    for k in range(KD):
                        nc.tensor.matmul(ps[:],
                                         lhsT=pw_tiles[m][:, k * 128:(k + 1) * 128],
                                         rhs=aT[:, k, tsl],
                                         start=(k == 0), stop=(k == KD - 1))
                    nc.vector.scalar_tensor_tensor(
                        xT[:, m, tsl], in0=ps[:], scalar=bp_sb[:, m:m + 1],
                        in1=xT[:, m, tsl], op0=OP.add, op1=OP.add)

            layer_norm_T(l1w_sb, l1b_sb, lnp_pool, True)
            if dbg and l == 0:
                nc.sync.dma_start(dbg['x1'], xT[:])
                nc.sync.dma_start(dbg['xb1'], xTb[:])
            # prefetch the gelu ACT table set while MLP1 matmuls run (the
            # xTb dep pins this after LN1's exp-family ops)
            nc.scalar.activation(scratch[:, 0:1], xTb[:, 0, T - 1:T], AF.Gelu)

            # ---- MLP1 (tch-major: LN1(tch1) overlaps MLP1(tch0)) ----
            for tch in range(NT):
                tsl = slice(tch * TCH, (tch + 1) * TCH)
                for m in range(FM):
                    w_sb = wpool6.tile([128, D], BF16, tag="w6")
                    nc.sync.dma_start(w_sb[:], w1[l, m])
                    ps = ps1()
                    for k in range(KD):
                        nc.tensor.matmul(ps[:],
                                         lhsT=w_sb[:, k * 128:(k + 1) * 128],
                                         rhs=xTb[:, k, tsl],
                                         start=(k == 0), stop=(k == KD - 1))
                    nc.scalar.activation(h[:, m, tsl], ps[:], AF.Gelu,
                                         bias=b1_sb[:, m:m + 1])
            # prefetch the sqrt table set while MLP2 matmuls run (dep on
            # the last gelu's output pins this after all gelus)
            nc.scalar.activation(scratch[:, 1:2], h[:, FM - 1, T - 1:T], AF.Sqrt)
            for tch in range(NT):
                tsl = slice(tch * TCH, (tch + 1) * TCH)
                for m in range(KD):
                    w2_sb = wpool24.tile([128, F], BF16, tag="w24")
                    nc.sync.dma_start(w2_sb[:], w2[l, m])
                    ps = ps1()
                    for k in range(FM):
                        nc.tensor.matmul(ps[:],
                                         lhsT=w2_sb[:, k * 128:(k + 1) * 128],
                                         rhs=h[:, k, tsl],
                                         start=(k == 0), stop=(k == FM - 1))
                    nc.vector.scalar_tensor_tensor(
                        xT[:, m, tsl], in0=ps[:], scalar=b2_sb[:, m:m + 1],
                        in1=xT[:, m, tsl], op0=OP.add, op1=OP.add)

            layer_norm_T(l2w_sb, l2b_sb, lnp_pool, l < n_layers - 1)
            # prefetch the exp table set for the next layer's attention
            nc.scalar.activation(scratch[:, 3:4], xTb[:, 0, T - 1:T], AF.Exp)
            if dbg and l == 0:
                nc.sync.dma_start(dbg['h'], h[:])
                nc.sync.dma_start(dbg['xbf'], xTb[:])
        lctx.close()

        # ---------------- Pooler ----------------
        with tc.tile_pool(name="poolp", bufs=1) as poolp:
            bpl_sb = poolp.tile([128, KD], F32)
            nc.sync.dma_start(bpl_sb[:], bpool.rearrange("(m p) -> p m", p=128))
            pool_sb = poolp.tile([128, KD, B_LOC], F32R)
            for m in range(KD):
                w_sb = poolp.tile([128, D], BF16, tag="wp", name="w_sb")
                nc.sync.dma_start(w_sb[:], wpool[m])
                ps = ps1()
                for k in range(KD):
                    first_tok = xTb[:, k, :].rearrange("p (b s) -> p b s", s=S)
                    nc.tensor.matmul(ps[:, :B_LOC],
                                     lhsT=w_sb[:, k * 128:(k + 1) * 128],
                                     rhs=first_tok[:, :, 0:1],
                                     start=(k == 0), stop=(k == KD - 1))
                nc.scalar.activation(pool_sb[:, m, :], ps[:, :B_LOC], AF.Tanh,
                                     bias=bpl_sb[:, m:m + 1])
            out_sb = poolp.tile([128, D], F32)
            for k in range(KD):
                pst = psum.tile([128, 128], F32R, tag="mm1", name="pst")
                nc.tensor.transpose(pst[:B_LOC, :], pool_sb[:, k, :], ident_sb[:])
                nc.vector.tensor_copy(out_sb[:B_LOC, k * 128:(k + 1) * 128],
                                      pst[:B_LOC, :])
            nc.sync.dma_start(out, out_sb[:B_LOC, :])


def _prep_host(inputs, n_layers=L):
    f32 = lambda a: np.asarray(a, dtype=np.float32)
    tokens = np.asarray(inputs["tokens"]).astype(np.int32)          # [16, 512]
    possent = f32(inputs["pos_emb"])[0] + f32(inputs["sent_emb"])[0, 0][None, :]

    nl = max(1, n_layers)
    # Fold layer-norm bias AND weight into the downstream GEMM (the device's
    # bf16 shadow xTb is (x-mu)*rstd only):
    #   W_eff[l]  = W[l] @ diag(prev_ln_w)
    #   b_eff[l]  = b[l] + W[l] @ prev_ln_b
    qkv_b = f32(inputs["qkv_b"]).copy()
    mlp_b1 = f32(inputs["mlp_b1"]).copy()
    pool_b = f32(inputs["pool_b"]).copy()
    emb_ln_w = f32(inputs["emb_ln_w"])
    emb_ln_b = f32(inputs["emb_ln_b"])
    ln1_w = f32(inputs["ln1_w"])
    ln1_b = f32(inputs["ln1_b"])
    ln2_w = f32(inputs["ln2_w"])
    ln2_b = f32(inputs["ln2_b"])

    qkv_w_eff = []
    mlp_w1_eff = []
    for l in range(nl):
        prev_w = emb_ln_w if l == 0 else ln2_w[l - 1]
        prev_b = emb_ln_b if l == 0 else ln2_b[l - 1]
        qw = f32(inputs["qkv_w"][l])
        qkv_b[l] = qkv_b[l] + qw @ prev_b
        qkv_w_eff.append(qw * prev_w[None, :])
        w1l = f32(inputs["mlp_w1"][l])
        mlp_b1[l] = mlp_b1[l] + w1l @ ln1_b[l]
        mlp_w1_eff.append(w1l * ln1_w[l][None, :])
    last_w = ln2_w[nl - 1] if n_layers >= 1 else emb_ln_w
    last_b = ln2_b[nl - 1] if n_layers >= 1 else emb_ln_b
    pw = f32(inputs["pool_w"])
    pool_b = pool_b + pw @ last_b
    pool_w_eff = pw * last_w[None, :]

    # Q,K weight tiles [nl, 12, 128, D]; V as wvT [nl, 128, KD, D]
    wqk = np.stack([_host_tile_weight(qkv_w_eff[l][:2 * D]) for l in range(nl)])
    wvT = np.stack([
        np.ascontiguousarray(
            qkv_w_eff[l][2 * D:].T.reshape(KD, 128, D).transpose(1, 0, 2)
        ).astype(ml_dtypes.bfloat16)
        for l in range(nl)])

    common = {
        "tok_emb": f32(inputs["tok_emb"]),
        "possent": possent.astype(np.float32),
        "embw": emb_ln_w,
        "embb": emb_ln_b,
        "wqk": wqk,
        "bqk": qkv_b[:nl, :2 * D],
        "wvT": wvT,
        "bv": qkv_b[:nl, 2 * D:],
        "wproj": np.stack([_host_tile_weight(f32(inputs["proj_w"][l]))
                           for l in range(nl)]),
        "bproj": f32(inputs["proj_b"])[:nl],
        "w1": np.stack([_host_tile_weight(mlp_w1_eff[l]) for l in range(nl)]),
        "b1": mlp_b1[:nl],
        "w2": np.stack([_host_tile_weight(f32(inputs["mlp_w2"][l]))
                        for l in range(nl)]),
        "b2": f32(inputs["mlp_b2"])[:nl],
        "ln1w": ln1_w[:nl],
        "ln1b": ln1_b[:nl],
        "ln2w": ln2_w[:nl],
        "ln2b": ln2_b[:nl],
        "wpool": _host_tile_weight(pool_w_eff),
        "bpool": pool_b,
        "ident": np.eye(128, dtype=np.float32),
        "ones": np.ones((128, 128), dtype=np.float32),
        "ones_bf": np.ones((128, 128), dtype=ml_dtypes.bfloat16),
    }
    in_maps = []
    for c in range(NCORES):
        m = dict(common)
        m["tokens"] = np.ascontiguousarray(
            tokens[c * B_LOC:(c + 1) * B_LOC].reshape(-1))
        in_maps.append(m)
    return in_maps


def kernel(**inputs) -> np.ndarray:
    global _CACHED_NC
    if _CACHED_NC is None:
        _CACHED_NC = build_nc(L)
    in_maps = _prep_host(inputs, L)
    res = run_bass_kernel_spmd(_CACHED_NC, in_maps,
                               core_ids=list(range(NCORES)), trace=False)
    return np.concatenate([res.results[c]["out"] for c in range(NCORES)], axis=0)
